# revision 16
# baseline (speedup 1.0000x reference)
"""Bass/Trainium2 kernel for nn_BranchingGNN (bipartite GNN message passing).

Strategy (8 NeuronCores, SPMD single NEFF, per-core data differs):
  - Nodes range-sharded: core i owns var rows [i*25000,(i+1)*25000) and con
    rows [i*12500,(i+1)*12500), padded to VR=25088 / CR=12800 storage rows.
  - Messages are linear, so  agg[d] = (sum_{e->d} h[src(e)]) @ W.T + deg(d)*b:
    sum raw h rows per destination FIRST, then apply the 64x64 weight.
  - Per direction, each core handles edges whose DESTINATION is in its range.
    Edges are bucketed by SOURCE core (8 buckets x NSUB sub-batches, so the
    int16 dma_gather index fits the per-core table chunk), sorted by source
    row. Each sub-batch: dma_gather (edge-major, HBM->SBUF, ~1 descriptor
    per edge) then dma_scatter_add (SBUF->HBM CCE add) into a zeroed DRAM
    aggregate table [n_dst, 64].
  - W pass: per 128-row tile of agg: PE transpose -> matmul with W.T,
    + h_old + deg*b, tanh -> chunk; chunks AllGather'd into the next
    direction's replicated table.
"""

import sys
import numpy as np
from contextlib import ExitStack
from dataclasses import dataclass

sys.path.insert(0, "/opt/trn_rl_repo")

# ---------------------------------------------------------------- config


@dataclass(frozen=True)
class Cfg:
    n_cores: int = 8
    nv: int = 200000          # total var nodes
    ncn: int = 100000         # total con nodes
    ne: int = 1200000         # total edges
    vf: int = 7
    cf: int = 5
    h: int = 64
    rounds: int = 2
    vr: int = 25088           # per-core var storage rows (mult of 512)
    cr: int = 12800           # per-core con storage rows (mult of 512)
    nsub: int = 2             # sub-batches per source bucket
    dma_scratch: int = 49152  # SWDGE ring bytes

    @property
    def v_own(self):
        return self.nv // self.n_cores

    @property
    def c_own(self):
        return self.ncn // self.n_cores

    @property
    def nb(self):             # sub-batches per direction
        return self.n_cores * self.nsub


FULL = Cfg()


def _wrap16(idx, L):
    """linear idx list (len L, mult of 128, padded with -1) -> [128, L//16]
    int16 tile: element i at [i%16, i//16], replicated over 8 groups."""
    a = np.full(L, -1, np.int16)
    a[:len(idx)] = idx
    a = a.reshape(-1, 16).T          # [16, L/16]
    return np.tile(a, (8, 1))        # [128, L/16]


# ---------------------------------------------------------------- host prep


def _rank_within_dest(db):
    """db sorted ascending -> per-element rank within its dest group."""
    starts = np.r_[0, np.flatnonzero(np.diff(db)) + 1]
    sizes = np.diff(np.r_[starts, len(db)])
    grp = np.repeat(np.arange(len(starts)), sizes)
    return np.arange(len(db)) - starts[grp]


MAXL = 1024  # HW dma_gather/scatter cap: 1024 indices per instruction


def _build_dir(src_glob, dst_glob, s_own, d_own, d_pad, n_cores):
    """Per-direction host prep: per core, dict (bucket, rank, piece) -> (g, d)
    with UNIQUE dests within each list (scatter-add RMW races otherwise) and
    len <= MAXL (HW per-instruction cap)."""
    per_core = []
    for i in range(n_cores):
        m = (dst_glob // d_own) == i
        s = src_glob[m]
        d = dst_glob[m]
        j = s // s_own                       # source core
        g = (s % s_own).astype(np.int64)     # gather idx within chunk j
        dl = (d % d_own).astype(np.int64)    # scatter idx (local dest row)
        subs = {}
        for b in range(n_cores):
            mb = j == b
            gb = g[mb]
            db = dl[mb]
            order = np.argsort(db, kind="stable")
            gb, db = gb[order], db[order]
            rank = _rank_within_dest(db)
            for k in range(int(rank.max(initial=-1)) + 1):
                sel = rank == k
                gk, dk = gb[sel], db[sel]
                o2 = np.argsort(gk, kind="stable")  # source-major locality
                gk, dk = gk[o2], dk[o2]
                for p in range(-(-len(gk) // MAXL)):
                    subs[(b, k, p)] = (gk[p * MAXL:(p + 1) * MAXL],
                                       dk[p * MAXL:(p + 1) * MAXL])
        per_core.append(subs)
    return per_core


def _pack_dir(per_core_subs, d_own, n_cores):
    """Unify (bucket, rank, piece) batches across cores (dummy edge where a
    core has none), pad to cross-core max (mult of 128) and pack into
    [128, sum(L)/16] int16 gidx/sidx arrays + [1, NB] counts per core.
    Returns (gidx, sidx, cnts, batches) with batches = tuple of (bucket, L)."""
    keys = sorted(set().union(*[set(p.keys()) for p in per_core_subs]))
    dummy = (np.array([0], np.int64), np.array([d_own], np.int64))
    Ls = []
    for key in keys:
        mx = max(len(p.get(key, dummy)[0]) for p in per_core_subs)
        Ls.append(max(128, -(-mx // 128) * 128))
    gidx, sidx, cnts = [], [], []
    for i in range(n_cores):
        gs, ss, cs = [], [], []
        for key, L in zip(keys, Ls):
            g, d = per_core_subs[i].get(key, dummy)
            if len(g) == 0:
                g, d = dummy
            assert g.max(initial=0) < 32768 and d.max(initial=0) < 32768
            gs.append(_wrap16(g.astype(np.int16), L))
            ss.append(_wrap16(d.astype(np.int16), L))
            cs.append(len(g))
        gidx.append(np.concatenate(gs, axis=1))
        sidx.append(np.concatenate(ss, axis=1))
        cnts.append(np.array(cs, np.int32).reshape(1, len(keys)))
    batches = tuple((int(k[0]), int(L)) for k, L in zip(keys, Ls))
    return gidx, sidx, cnts, batches


def prep_inputs(inputs, cfg: Cfg):
    """Full numpy preprocessing -> (list of per-core input dicts, b_ro,
    static sub-batch sizes per direction)."""
    c = cfg
    ev = np.asarray(inputs["edge_var"]).astype(np.int64)
    ec = np.asarray(inputs["edge_con"]).astype(np.int64)
    xv = np.asarray(inputs["var_features"], np.float32)
    xc = np.asarray(inputs["con_features"], np.float32)

    # padded, transposed feature arrays
    xv_t = np.zeros((c.vf, c.n_cores * c.vr), np.float32)
    xc_t = np.zeros((c.cf, c.n_cores * c.cr), np.float32)
    lv = np.arange(c.nv)
    lc = np.arange(c.ncn)
    xv_t[:, (lv // c.v_own) * c.vr + lv % c.v_own] = xv.T
    xc_t[:, (lc // c.c_own) * c.cr + lc % c.c_own] = xc.T

    deg_c = np.bincount(ec, minlength=c.ncn).astype(np.float32)
    deg_v = np.bincount(ev, minlength=c.nv).astype(np.float32)
    deg_con = np.zeros((c.n_cores, c.cr), np.float32)
    deg_var = np.zeros((c.n_cores, c.vr), np.float32)
    deg_con[lc // c.c_own, lc % c.c_own] = deg_c
    deg_var[lv // c.v_own, lv % c.v_own] = deg_v

    subs_v2c = _build_dir(ev, ec, c.v_own, c.c_own, c.cr, c.n_cores)
    subs_c2v = _build_dir(ec, ev, c.c_own, c.v_own, c.vr, c.n_cores)
    g_v2c, s_v2c, n_v2c, L_v2c = _pack_dir(subs_v2c, c.c_own, c.n_cores)
    g_c2v, s_c2v, n_c2v, L_c2v = _pack_dir(subs_c2v, c.v_own, c.n_cores)
    assert c.c_own < c.cr and c.v_own < c.vr  # dummy-edge rows must exist

    per_core = []
    for i in range(c.n_cores):
        per_core.append(dict(
            xv_t=np.ascontiguousarray(xv_t[:, i * c.vr:(i + 1) * c.vr]),
            xc_t=np.ascontiguousarray(xc_t[:, i * c.cr:(i + 1) * c.cr]),
            gidx_v2c=g_v2c[i], sidx_v2c=s_v2c[i], cnt_v2c=n_v2c[i],
            gidx_c2v=g_c2v[i], sidx_c2v=s_c2v[i], cnt_c2v=n_c2v[i],
            deg_con=deg_con[i].reshape(c.cr, 1),
            deg_var=deg_var[i].reshape(c.vr, 1),
        ))

    # weights (shared across cores)
    w = {}
    w["w1v_t"] = np.ascontiguousarray(np.asarray(inputs["W_ve1"], np.float32).T)
    w["w2v_t"] = np.ascontiguousarray(np.asarray(inputs["W_ve2"], np.float32).T)
    w["b1v"] = np.asarray(inputs["b_ve1"], np.float32).reshape(c.h, 1)
    w["b2v"] = np.asarray(inputs["b_ve2"], np.float32).reshape(c.h, 1)
    w["w1c_t"] = np.ascontiguousarray(np.asarray(inputs["W_ce1"], np.float32).T)
    w["w2c_t"] = np.ascontiguousarray(np.asarray(inputs["W_ce2"], np.float32).T)
    w["b1c"] = np.asarray(inputs["b_ce1"], np.float32).reshape(c.h, 1)
    w["b2c"] = np.asarray(inputs["b_ce2"], np.float32).reshape(c.h, 1)
    for r in range(c.rounds):
        w[f"wt_v2c_{r}"] = np.ascontiguousarray(
            np.asarray(inputs["W_v2c"], np.float32)[r].T)
        w[f"wt_c2v_{r}"] = np.ascontiguousarray(
            np.asarray(inputs["W_c2v"], np.float32)[r].T)
        w[f"b_v2c_{r}"] = np.broadcast_to(
            np.asarray(inputs["b_v2c"], np.float32)[r], (128, c.h)).copy()
        w[f"b_c2v_{r}"] = np.broadcast_to(
            np.asarray(inputs["b_c2v"], np.float32)[r], (128, c.h)).copy()
    w["wro_rep"] = np.ascontiguousarray(np.tile(
        np.asarray(inputs["W_ro"], np.float32).reshape(1, c.h), (128, 1)))
    b_ro = float(np.asarray(inputs["b_ro"]).reshape(-1)[0])

    for pc in per_core:
        pc.update(w)
    return per_core, b_ro, (L_v2c, L_c2v)


# ---------------------------------------------------------------- builder


def build_nc(cfg: Cfg, b_ro: float, Ls):
    from concourse import bass, mybir, tile, library_config
    import concourse.bacc as bacc
    from concourse.masks import make_identity

    c = cfg
    L_v2c, L_c2v = Ls
    f32 = mybir.dt.float32
    i32 = mybir.dt.int32
    i16 = mybir.dt.int16
    H = c.h

    nc = bacc.Bacc("TRN2", target_bir_lowering=False, debug=False,
                   num_devices=c.n_cores,
                   dynamic_dma_scratch_size=c.dma_scratch)

    def inp(name, shape, dt=f32):
        return nc.dram_tensor(name, list(shape), dt, kind="ExternalInput").ap()

    xv_t = inp("xv_t", [c.vf, c.vr])
    xc_t = inp("xc_t", [c.cf, c.cr])
    tot_v2c = sum(l for _, l in L_v2c)
    tot_c2v = sum(l for _, l in L_c2v)
    gidx = {"v2c": inp("gidx_v2c", [128, tot_v2c // 16], i16),
            "c2v": inp("gidx_c2v", [128, tot_c2v // 16], i16)}
    sidx = {"v2c": inp("sidx_v2c", [128, tot_v2c // 16], i16),
            "c2v": inp("sidx_c2v", [128, tot_c2v // 16], i16)}
    cnt = {"v2c": inp("cnt_v2c", [1, len(L_v2c)], i32),
           "c2v": inp("cnt_c2v", [1, len(L_c2v)], i32)}
    deg = {"v2c": inp("deg_con", [c.cr, 1]),
           "c2v": inp("deg_var", [c.vr, 1])}
    w1v_t = inp("w1v_t", [c.vf, H]); w2v_t = inp("w2v_t", [H, H])
    b1v = inp("b1v", [H, 1]); b2v = inp("b2v", [H, 1])
    w1c_t = inp("w1c_t", [c.cf, H]); w2c_t = inp("w2c_t", [H, H])
    b1c = inp("b1c", [H, 1]); b2c = inp("b2c", [H, 1])
    wts = {}
    for r in range(c.rounds):
        wts[("v2c", r)] = (inp(f"wt_v2c_{r}", [H, H]), inp(f"b_v2c_{r}", [128, H]))
        wts[("c2v", r)] = (inp(f"wt_c2v_{r}", [H, H]), inp(f"b_c2v_{r}", [128, H]))
    wro_rep = inp("wro_rep", [128, H])
    scores = nc.dram_tensor("scores", [c.vr], f32, kind="ExternalOutput").ap()

    groups = [list(range(c.n_cores))]
    nvp = c.n_cores * c.vr
    ncp = c.n_cores * c.cr

    with tile.TileContext(nc) as tc:
        with ExitStack() as ctx:
            dram = ctx.enter_context(tc.tile_pool(name="dram", bufs=1, space="DRAM"))
            cpool = ctx.enter_context(tc.tile_pool(name="consts", bufs=1))
            sb = ctx.enter_context(tc.tile_pool(name="sb", bufs=3))
            sb2 = ctx.enter_context(tc.tile_pool(name="sb2", bufs=2))
            gpool = ctx.enter_context(tc.tile_pool(name="gpool", bufs=2))
            ps = ctx.enter_context(tc.tile_pool(name="ps", bufs=2, space="PSUM"))

            var_tab = [dram.tile([nvp, H], f32, name=f"var_tab{r}",
                                 tag=f"var_tab{r}") for r in range(c.rounds)]
            con_tab = [dram.tile([ncp, H], f32, name=f"con_tab{r}",
                                 tag=f"con_tab{r}") for r in range(c.rounds)]
            chunk_var = [dram.tile([c.vr, H], f32, name=f"chunk_var{j}",
                                   tag=f"chunk_var{j}") for j in range(2)]
            chunk_con = [dram.tile([c.cr, H], f32, name=f"chunk_con{j}",
                                   tag=f"chunk_con{j}") for j in range(3)]
            A = 4  # alternating scatter-accumulate buffers
            agg_con = [dram.tile([A * c.cr, H], f32, name=f"agg_con{r}",
                                 tag=f"agg_con{r}") for r in range(c.rounds)]
            agg_var = [dram.tile([A * c.vr, H], f32, name=f"agg_var{r}",
                                 tag=f"agg_var{r}") for r in range(c.rounds)]

            ident = cpool.tile([128, 128], f32, name="ident", tag="ident")
            make_identity(nc, ident)
            zt = cpool.tile([128, 1280], f32, name="zt", tag="zt")
            nc.vector.memset(zt[:], 0.0)

            nc.gpsimd.load_library(library_config.mlp)
            cnt_regs = [nc.gpsimd.alloc_register(f"cnt{k}") for k in range(16)]

            def zero_dram(t, rows):
                assert rows % 128 == 0
                z = 0
                while z < rows:
                    k = min(20, (rows - z) // 128)
                    nc.sync.dma_start(
                        t[z:z + k * 128, :].rearrange(
                            "(q p) f -> p q f", p=128),
                        zt[:, :k * H].rearrange("p (q f) -> p q f", f=H))
                    z += k * 128

            for r in range(c.rounds):
                zero_dram(agg_con[r], A * c.cr)
                zero_dram(agg_var[r], A * c.vr)

            # ---------------- encoder: x_t [F, rows] -> chunk [rows, H]
            def encode(x_t, F, rows, w1, b1, w2, b2, out_chunk):
                w1_sb = sb2.tile([F, H], f32, name="w1_sb", tag="encw1")
                nc.sync.dma_start(w1_sb[:], w1[:])
                w2_sb = sb2.tile([H, H], f32, name="w2_sb", tag="encw2")
                nc.sync.dma_start(w2_sb[:], w2[:])
                b1_sb = sb2.tile([H, 1], f32, name="b1_sb", tag="encb1")
                nc.sync.dma_start(b1_sb[:], b1[:])
                b2_sb = sb2.tile([H, 1], f32, name="b2_sb", tag="encb2")
                nc.sync.dma_start(b2_sb[:], b2[:])
                for t in range(rows // 512):
                    xt = sb.tile([F, 512], f32, name="xt", tag="enc_xt")
                    nc.sync.dma_start(xt[:], x_t[:, t * 512:(t + 1) * 512])
                    p1 = ps.tile([H, 512], f32, name="p1", tag="mmA")
                    nc.tensor.matmul(p1[:], lhsT=w1_sb[:], rhs=xt[:],
                                     start=True, stop=True)
                    t1 = sb.tile([H, 512], f32, name="t1", tag="enc_t1")
                    nc.scalar.activation(t1[:], p1[:],
                                         mybir.ActivationFunctionType.Tanh,
                                         bias=b1_sb[:, :])
                    p2 = ps.tile([H, 512], f32, name="p2", tag="mmA")
                    nc.tensor.matmul(p2[:], lhsT=w2_sb[:], rhs=t1[:],
                                     start=True, stop=True)
                    h2 = sb.tile([H, 512], f32, name="h2", tag="enc_h2")
                    nc.vector.tensor_scalar_add(h2[:], p2[:], b2_sb[:, :])
                    hn = sb.tile([128, 4 * H], f32, name="hn", tag="enc_hn")
                    for q in range(4):
                        pt = ps.tile([128, H], f32, name="pt", tag="trp")
                        nc.tensor.transpose(
                            pt[:], h2[:, q * 128:(q + 1) * 128], ident[:H, :H])
                        nc.scalar.activation(
                            hn[:, q * H:(q + 1) * H], pt[:],
                            mybir.ActivationFunctionType.Copy)
                    nc.sync.dma_start(
                        out_chunk[t * 512:(t + 1) * 512, :].rearrange(
                            "(q p) f -> p q f", p=128), hn[:])

            encode(xv_t, c.vf, c.vr, w1v_t, b1v, w2v_t, b2v, chunk_var[0])
            encode(xc_t, c.cf, c.cr, w1c_t, b1c, w2c_t, b2c, chunk_con[0])

            def allgather(chunk, tab, rows_total):
                nc.gpsimd.collective_compute(
                    "AllGather", mybir.AluOpType.bypass,
                    replica_groups=groups,
                    ins=[chunk[:, :]],
                    outs=[tab[0:rows_total, :]],
                )

            allgather(chunk_var[0], var_tab[0], nvp)

            # ---------------- one message-passing direction
            def msg_pass(d, r, src_tab, src_rows, agg, n_dst, L, w_t, b_rep,
                         h_old_chunk, out_chunk, readout=None):
                wt_sb = sb2.tile([H, H], f32, name="wt_sb", tag="msg_wt")
                nc.sync.dma_start(wt_sb[:], w_t[:])
                brep_sb = sb2.tile([128, H], f32, name="brep_sb", tag="msg_brep")
                nc.sync.dma_start(brep_sb[:], b_rep[:])
                cnt_sb = cpool.tile([1, len(L)], i32, name=f"cnt_{d}{r}",
                                    tag=f"cnt_{d}{r}")
                nc.sync.dma_start(cnt_sb[:], cnt[d][:])

                off = 0
                for s, (j, Lb) in enumerate(L):
                    gi = sb.tile([128, Lb // 16], i16, name="gi", tag="gi")
                    nc.sync.dma_start(
                        gi[:], gidx[d][:, off // 16:(off + Lb) // 16])
                    si = sb.tile([128, Lb // 16], i16, name="si", tag="si")
                    nc.sync.dma_start(
                        si[:], sidx[d][:, off // 16:(off + Lb) // 16])
                    reg = cnt_regs[s % 16]
                    nc.gpsimd.reg_load(reg, cnt_sb[0:1, s:s + 1])
                    gb = gpool.tile([128, Lb // 128, H], f32, name="gb",
                                    tag="gb")
                    nc.gpsimd.dma_gather(
                        gb[:], src_tab[j * src_rows:(j + 1) * src_rows, :],
                        gi[:], Lb, reg, H)
                    a = s % A
                    nc.gpsimd.dma_scatter_add(
                        agg[a * n_dst:(a + 1) * n_dst, :], gb[:], si[:],
                        Lb, reg, H)
                    off += Lb

                # W pass over agg tiles (reduce the A accumulators)
                agg_v = agg[0:A * n_dst, :].rearrange("(a q) f -> q a f",
                                                      a=A)
                for t in range(n_dst // 128):
                    base = t * 128
                    asb4 = sb.tile([128, A, H], f32, name="asb4", tag="asb4")
                    nc.sync.dma_start(asb4[:], agg_v[base:base + 128, :, :])
                    asb = sb.tile([128, H], f32, name="asb", tag="asb")
                    nc.vector.tensor_reduce(
                        asb[:],
                        asb4[:].rearrange("p a f -> p f a"),
                        axis=mybir.AxisListType.X, op=mybir.AluOpType.add)
                    ptr = ps.tile([H, 128], f32, name="ptr", tag="trp")
                    nc.tensor.transpose(ptr[:], asb[:], ident[:, :])
                    gfm = sb.tile([H, 128], f32, name="gfm", tag="gfm")
                    nc.scalar.activation(
                        gfm[:], ptr[:], mybir.ActivationFunctionType.Copy)
                    pst = ps.tile([128, H], f32, name="pst", tag="agg")
                    nc.tensor.matmul(pst[:], lhsT=gfm[:], rhs=wt_sb[:],
                                     start=True, stop=True)
                    hold = sb.tile([128, H], f32, name="hold", tag="hold")
                    nc.sync.dma_start(hold[:], h_old_chunk[base:base + 128, :])
                    degc = sb.tile([128, 1], f32, name="degc", tag="degc")
                    nc.sync.dma_start(degc[:], deg[d][base:base + 128, :])
                    hk = sb.tile([128, H], f32, name="hk", tag="hk")
                    nc.vector.tensor_scalar_mul(
                        hk[:], brep_sb[:], degc[:, 0:1])
                    nc.vector.tensor_add(hk[:], hk[:], hold[:])
                    nc.vector.tensor_add(hk[:], hk[:], pst[:])
                    nc.scalar.activation(
                        hk[:], hk[:], mybir.ActivationFunctionType.Tanh)
                    if out_chunk is not None:
                        nc.sync.dma_start(out_chunk[base:base + 128, :], hk[:])
                    if readout is not None:
                        wro_sb, sc_sb = readout
                        m = sb.tile([128, H], f32, name="m", tag="romul")
                        nc.vector.tensor_mul(m[:], hk[:], wro_sb[:])
                        nc.vector.tensor_reduce(
                            sc_sb[:, t:t + 1],
                            m[:].rearrange("p (o f) -> p o f", f=H),
                            axis=mybir.AxisListType.X,
                            op=mybir.AluOpType.add)

            wro_sb = cpool.tile([128, H], f32, name="wro_sb", tag="wro_sb")
            nc.sync.dma_start(wro_sb[:], wro_rep[:])
            sc_sb = cpool.tile([128, c.vr // 128], f32, name="sc_sb",
                               tag="sc_sb")

            seq = []
            for r in range(c.rounds):
                seq.append(("v2c", r))
                seq.append(("c2v", r))

            con_state = chunk_con[0]
            var_state = chunk_var[0]
            for (d, r) in seq:
                last = (d, r) == seq[-1]
                w_t, b_rep = wts[(d, r)]
                if d == "v2c":
                    out = chunk_con[r + 1]
                    msg_pass(d, r, var_tab[r], c.vr, agg_con[r], c.cr, L_v2c,
                             w_t, b_rep, con_state, out)
                    allgather(out, con_tab[r], ncp)
                    con_state = out
                else:
                    out = None if last else chunk_var[r + 1]
                    msg_pass(d, r, con_tab[r], c.cr, agg_var[r], c.vr, L_c2v,
                             w_t, b_rep, var_state, out,
                             readout=(wro_sb, sc_sb) if last else None)
                    if not last:
                        allgather(out, var_tab[r + 1], nvp)
                        var_state = out

            # readout epilogue: sc_sb [128, ntiles] -> scores [vr]
            nt = c.vr // 128
            nc.vector.tensor_scalar_add(sc_sb[:], sc_sb[:], float(b_ro))
            for half in range(2):
                w2 = nt // 2
                pt = ps.tile([w2, 128], f32, name="pt_ro", tag="trp")
                nc.tensor.transpose(
                    pt[:], sc_sb[:, half * w2:(half + 1) * w2], ident[:, :])
                so = sb.tile([w2, 128], f32, name="so", tag="so")
                nc.scalar.activation(
                    so[:], pt[:], mybir.ActivationFunctionType.Copy)
                nc.sync.dma_start(
                    scores[half * w2 * 128:(half + 1) * w2 * 128].rearrange(
                        "(q p) -> q p", p=128), so[:])

    nc.compile()
    return nc


# ---------------------------------------------------------------- runner

_CACHE = {}


def _get_nc(cfg, b_ro, Ls):
    key = (cfg, round(b_ro, 10), Ls)
    if key not in _CACHE:
        _CACHE[key] = build_nc(cfg, b_ro, Ls)
    return _CACHE[key]


def unpermute(res_list, cfg: Cfg):
    return np.concatenate(
        [r["scores"][:cfg.v_own] for r in res_list]).astype(np.float32)


def run(inputs, cfg: Cfg = FULL, trace=False):
    from concourse import bass_utils
    per_core, b_ro, Ls = prep_inputs(inputs, cfg)
    nc = _get_nc(cfg, b_ro, Ls)
    res = bass_utils.run_bass_kernel_spmd(
        nc, per_core, core_ids=list(range(cfg.n_cores)), trace=trace)
    return unpermute(res.results, cfg), res


def kernel(**inputs) -> np.ndarray:
    out, _ = run(inputs, FULL)
    return out
